# revision 11
# baseline (speedup 1.0000x reference)
"""Trainium2 Bass kernel for nn_H_MAx_C_MaxAtt (pooling attention module).

Reference computation (per sample n):
    x_h[c,h]  = mean_w x + max_w x
    y[m,h]    = conv1_w @ x_h + b ; BN ; h_swish
    a_h[c,h]  = sigmoid(conv_h_w @ y + conv_h_b)
    g[c]      = mean_hw x + max_hw x
    ca[c]     = sigmoid(fc_w @ g + fc_b)
    out       = x * a_h[:, :, None] * ca[:, None, None]

Strategy: data-parallel over batch N (16 samples / 8 cores = 2 per core).
Each sample's x [256, 128, 128] f32 is held in SBUF as 4 tiles
[128c, 64h, 128w] (32 KiB/partition each) so x is read from HBM exactly
once and the output written exactly once (memory roofline: 64 MiB/core).

Engine split per tile:
  - DVE: reduce_sum / reduce_max over w (the two big reductions)
  - PE : all the tiny matmuls (conv1, conv_h, fc)
  - ACT: sigmoids + the big broadcast multiply x * a2[c,h] as a loop of
    per-h activation(Copy, scale=a2[:,h]) ops (per-partition scale)
"""

import sys

if "/opt/trn_rl_repo" not in sys.path:
    sys.path.insert(0, "/opt/trn_rl_repo")

from contextlib import ExitStack

import numpy as np

import concourse.bass as bass
import concourse.bacc as bacc
import concourse.tile as tile
from concourse import mybir
from concourse.bass_utils import run_bass_kernel_spmd

F32 = mybir.dt.float32
AF = mybir.ActivationFunctionType
ALU = mybir.AluOpType
AX = mybir.AxisListType

N, C, H, W = 16, 256, 128, 128
MIP = 8
N_CORES = 8
NPC = N // N_CORES  # samples per core
CH = C // 128       # channel chunks of 128
HH = 2              # h chunks
HT = H // HH        # h per tile

EPS = 1e-5


def _build_program(repeats: int = 1) -> bass.Bass:
    nc = bacc.Bacc("TRN2", target_bir_lowering=False, debug=False)

    xd = nc.dram_tensor("x", [NPC, C, H, W], F32, kind="ExternalInput").ap()
    w1t_d = nc.dram_tensor("w1t", [CH, 128, MIP], F32, kind="ExternalInput").ap()
    bns_d = nc.dram_tensor("bns", [MIP, 1], F32, kind="ExternalInput").ap()
    bnb_d = nc.dram_tensor("bnb", [MIP, 1], F32, kind="ExternalInput").ap()
    wht_d = nc.dram_tensor("wht6", [MIP, C], F32, kind="ExternalInput").ap()
    chb_d = nc.dram_tensor("chb", [CH, 128, 1], F32, kind="ExternalInput").ap()
    fct_d = nc.dram_tensor("fct", [CH, 128, C], F32, kind="ExternalInput").ap()
    fcb_d = nc.dram_tensor("fcb", [CH, 128, 1], F32, kind="ExternalInput").ap()
    outd = nc.dram_tensor("out", [NPC, C, H, W], F32, kind="ExternalOutput").ap()

    with tile.TileContext(nc) as tc, ExitStack() as ctx:
        consts = ctx.enter_context(tc.tile_pool(name="consts", bufs=1))
        xt_pool = ctx.enter_context(tc.tile_pool(name="xt", bufs=5))
        small = ctx.enter_context(tc.tile_pool(name="small", bufs=3))
        psum = ctx.enter_context(tc.tile_pool(name="psum", bufs=2, space="PSUM"))

        # ---- load replicated parameters once ----
        w1t_sb = consts.tile([128, CH, MIP], F32, tag="w1t")
        fct_sb = consts.tile([128, CH, C], F32, tag="fct")
        chb_sb = consts.tile([128, CH, 1], F32, tag="chb")
        fcb_sb = consts.tile([128, CH, 1], F32, tag="fcb")
        for ch in range(CH):
            nc.sync.dma_start(out=w1t_sb[:, ch, :], in_=w1t_d[ch])
            nc.sync.dma_start(out=fct_sb[:, ch, :], in_=fct_d[ch])
            nc.sync.dma_start(out=chb_sb[:, ch, :], in_=chb_d[ch])
            nc.sync.dma_start(out=fcb_sb[:, ch, :], in_=fcb_d[ch])
        wht_sb = consts.tile([MIP, C], F32, tag="wht")
        nc.sync.dma_start(out=wht_sb[:], in_=wht_d[:])
        bns_sb = consts.tile([MIP, 1], F32, tag="bns")
        nc.sync.dma_start(out=bns_sb[:], in_=bns_d[:])
        bnb_sb = consts.tile([MIP, 1], F32, tag="bnb")
        nc.sync.dma_start(out=bnb_sb[:], in_=bnb_d[:])
        three_sb = consts.tile([MIP, 1], F32, tag="three")
        nc.vector.memset(three_sb[:], 3.0)

        for s in [s for _ in range(repeats) for s in range(NPC)]:
            # ---- load sample, 4 tiles of [128c, HT, 128w] ----
            xts = {}
            for ch in range(CH):
                for hh in range(HH):
                    xt = xt_pool.tile([128, HT, W], F32, tag="xt")
                    nc.sync.dma_start(
                        out=xt[:],
                        in_=xd[s, ch * 128:(ch + 1) * 128, hh * HT:(hh + 1) * HT, :],
                    )
                    xts[ch, hh] = xt

            # ---- width pools (DVE) ----
            xh_sum, xh_max = {}, {}
            for ch in range(CH):
                hs = small.tile([128, H], F32, tag="xh_sum")
                hm = small.tile([128, H], F32, tag="xh_max")
                for hh in range(HH):
                    sl = slice(hh * HT, (hh + 1) * HT)
                    nc.vector.reduce_sum(hs[:, sl], xts[ch, hh][:], axis=AX.X)
                    nc.vector.reduce_max(hm[:, sl], xts[ch, hh][:], axis=AX.X)
                xh_sum[ch], xh_max[ch] = hs, hm

            # x_h = wsum/W + wmax ; g = gsum/(H*W) + gmax
            xh, g = {}, {}
            for ch in range(CH):
                t = small.tile([128, H], F32, tag="xh")
                nc.vector.scalar_tensor_tensor(
                    t[:], in0=xh_sum[ch][:], scalar=1.0 / W, in1=xh_max[ch][:],
                    op0=ALU.mult, op1=ALU.add,
                )
                xh[ch] = t
                gs = small.tile([128, 1], F32, tag="gsum")
                gm = small.tile([128, 1], F32, tag="gmax")
                gt = small.tile([128, 1], F32, tag="g")
                nc.vector.reduce_sum(gs[:], xh_sum[ch][:], axis=AX.X)
                nc.vector.reduce_max(gm[:], xh_max[ch][:], axis=AX.X)
                nc.vector.scalar_tensor_tensor(
                    gt[:], in0=gs[:], scalar=1.0 / (H * W), in1=gm[:],
                    op0=ALU.mult, op1=ALU.add,
                )
                g[ch] = gt

            # ---- conv1 + BN + h_swish (tiny, [MIP, H]) ----
            y_ps = psum.tile([MIP, H], F32, tag="y_ps")
            for ch in range(CH):
                nc.tensor.matmul(
                    y_ps[:], lhsT=w1t_sb[:, ch, :], rhs=xh[ch][:],
                    start=(ch == 0), stop=(ch == CH - 1),
                )
            y_sb = small.tile([MIP, H], F32, tag="y")
            nc.scalar.activation(
                y_sb[:], y_ps[:], AF.Identity, bias=bnb_sb[:], scale=bns_sb[:],
            )
            t_sb = small.tile([MIP, H], F32, tag="t")
            nc.scalar.activation(t_sb[:], y_sb[:], AF.Relu, bias=three_sb[:])
            nc.vector.tensor_scalar_min(t_sb[:], t_sb[:], 6.0)
            y2 = small.tile([MIP, H], F32, tag="y2")
            nc.vector.tensor_mul(y2[:], y_sb[:], t_sb[:])

            # ---- a_h = sigmoid(conv_h/6 @ y2 + b) ; per-chunk [128, H] ----
            a2 = {}
            for ch in range(CH):
                aps = psum.tile([128, H], F32, tag="a_ps")
                nc.tensor.matmul(
                    aps[:], lhsT=wht_sb[:, ch * 128:(ch + 1) * 128], rhs=y2[:],
                )
                a = small.tile([128, H], F32, tag="a")
                nc.scalar.activation(a[:], aps[:], AF.Sigmoid, bias=chb_sb[:, ch, :])
                a2[ch] = a

            # ---- ca = sigmoid(fc @ g + b) ; per-chunk [128, 1] ----
            for ch in range(CH):
                cps = psum.tile([128, 1], F32, tag="ca_ps")
                for j in range(CH):
                    nc.tensor.matmul(
                        cps[:], lhsT=fct_sb[:, j, ch * 128:(ch + 1) * 128],
                        rhs=g[j][:], start=(j == 0), stop=(j == CH - 1),
                    )
                casb = small.tile([128, 1], F32, tag="ca")
                nc.scalar.activation(casb[:], cps[:], AF.Sigmoid, bias=fcb_sb[:, ch, :])
                # fold channel attention into a_h: a2 = a_h * ca  (per-partition)
                nc.vector.tensor_scalar_mul(a2[ch][:], a2[ch][:], casb[:])

            # ---- out = x * a2[c,h]  (ACT, per-h scale) + writeout ----
            for ch in range(CH):
                for hh in range(HH):
                    xt = xts[ch, hh]
                    for j in range(HT):
                        h = hh * HT + j
                        nc.scalar.mul(
                            xt[:, j, :], xt[:, j, :], a2[ch][:, h:h + 1],
                        )
                    nc.sync.dma_start(
                        out=outd[s, ch * 128:(ch + 1) * 128, hh * HT:(hh + 1) * HT, :],
                        in_=xt[:],
                    )
    nc.compile()
    return nc


_NC_CACHE = {}


def _get_program(repeats: int = 1) -> bass.Bass:
    if repeats not in _NC_CACHE:
        _NC_CACHE[repeats] = _build_program(repeats)
    return _NC_CACHE[repeats]


def _prep_in_maps(inputs: dict) -> list:
    f = lambda a: np.ascontiguousarray(np.asarray(a, dtype=np.float32))
    x = f(inputs["x"])
    conv1_w = f(inputs["conv1_w"])
    conv1_b = f(inputs["conv1_b"])
    bn_gamma = f(inputs["bn_gamma"])
    bn_beta = f(inputs["bn_beta"])
    bn_mean = f(inputs["bn_mean"])
    bn_var = f(inputs["bn_var"])
    conv_h_w = f(inputs["conv_h_w"])
    conv_h_b = f(inputs["conv_h_b"])
    fc_w = f(inputs["fc_w"])
    fc_b = f(inputs["fc_b"])

    # Host-side folds (all tiny):
    #   BN(y) = y*bns + bnb  with bns = gamma/sqrt(var+eps), bnb = beta - mean*bns
    #   ... but conv1 bias must be applied before BN: fold conv1_b into bnb.
    bns = bn_gamma / np.sqrt(bn_var + EPS)
    bnb = bn_beta + (conv1_b - bn_mean) * bns
    w1t = np.ascontiguousarray(conv1_w.T).reshape(CH, 128, MIP)
    wht6 = np.ascontiguousarray(conv_h_w.T) / 6.0           # [MIP, C]
    chb = conv_h_b.reshape(CH, 128, 1)
    fct = np.ascontiguousarray(fc_w.T).reshape(CH, 128, C)  # fct[j, c] = fc_w[c, j]
    fcb = fc_b.reshape(CH, 128, 1)

    shared = {
        "w1t": w1t.astype(np.float32),
        "bns": bns.reshape(MIP, 1).astype(np.float32),
        "bnb": bnb.reshape(MIP, 1).astype(np.float32),
        "wht6": wht6.astype(np.float32),
        "chb": chb.astype(np.float32),
        "fct": fct.astype(np.float32),
        "fcb": fcb.astype(np.float32),
    }
    return [
        {"x": np.ascontiguousarray(x[i * NPC:(i + 1) * NPC]), **shared}
        for i in range(N_CORES)
    ]


def _run(inputs: dict, trace: bool = False, repeats: int = 1):
    nc = _get_program(repeats)
    in_maps = _prep_in_maps(inputs)
    res = run_bass_kernel_spmd(nc, in_maps, list(range(N_CORES)), trace=trace)
    out = np.concatenate([res.results[i]["out"] for i in range(N_CORES)], axis=0)
    return out, res


def kernel(**inputs) -> np.ndarray:
    out, _ = _run(inputs)
    return out


if __name__ == "__main__":
    # smoke test with random data through the simulator-free path
    rng = np.random.default_rng(0)
    ins = {
        "x": rng.standard_normal((N, C, H, W), dtype=np.float32),
        "conv1_w": rng.standard_normal((MIP, C), dtype=np.float32) * 0.05,
        "conv1_b": rng.standard_normal((MIP,), dtype=np.float32) * 0.05,
        "bn_gamma": np.ones((MIP,), np.float32),
        "bn_beta": np.zeros((MIP,), np.float32),
        "bn_mean": rng.standard_normal((MIP,), dtype=np.float32) * 0.1,
        "bn_var": rng.random((MIP,), dtype=np.float32) * 0.5 + 0.5,
        "conv_h_w": rng.standard_normal((C, MIP), dtype=np.float32) * 0.05,
        "conv_h_b": rng.standard_normal((C,), dtype=np.float32) * 0.05,
        "fc_w": rng.standard_normal((C, C), dtype=np.float32) * 0.05,
        "fc_b": rng.standard_normal((C,), dtype=np.float32) * 0.05,
    }
    out = kernel(**ins)
    print("out", out.shape, out.dtype, float(np.abs(out).max()))


# revision 18
# speedup vs baseline: 518.8846x; 518.8846x over previous
"""Trainium2 Bass kernel for nn_H_MAx_C_MaxAtt (pooling attention module).

Reference computation (per sample n):
    x_h[c,h]  = mean_w x + max_w x
    y[m,h]    = conv1_w @ x_h + b ; BN ; h_swish
    a_h[c,h]  = sigmoid(conv_h_w @ y + conv_h_b)
    g[c]      = mean_hw x + max_hw x
    ca[c]     = sigmoid(fc_w @ g + fc_b)
    out       = x * a_h[:, :, None] * ca[:, None, None]

Strategy: data-parallel over batch N (16 samples / 8 cores = 2 per core).
Each sample's x [256, 128, 128] f32 is held in SBUF as 4 tiles
[128c, 64h, 128w] (32 KiB/partition each) so x is read from HBM exactly
once and the output written exactly once (memory roofline: 64 MiB/core).

Engine split per tile:
  - DVE: reduce_sum / reduce_max over w (the two big reductions)
  - PE : all the tiny matmuls (conv1, conv_h, fc)
  - ACT: sigmoids + the big broadcast multiply x * a2[c,h] as a loop of
    per-h activation(Copy, scale=a2[:,h]) ops (per-partition scale)
"""

import sys

if "/opt/trn_rl_repo" not in sys.path:
    sys.path.insert(0, "/opt/trn_rl_repo")

from contextlib import ExitStack

import numpy as np

import concourse.bass as bass
import concourse.bacc as bacc
import concourse.tile as tile
from concourse import mybir
from concourse.bass_utils import run_bass_kernel_spmd

F32 = mybir.dt.float32
AF = mybir.ActivationFunctionType
ALU = mybir.AluOpType
AX = mybir.AxisListType

N, C, H, W = 16, 256, 128, 128
MIP = 8
N_CORES = 8
NPC = N // N_CORES  # samples per core
CH = C // 128       # channel chunks of 128
HH = 8              # h chunks
HT = H // HH        # h per tile
MUL_DVE_FRAC = {0: 0.0, 1: 0.75}  # sample -> fraction of h-row muls on DVE
XT_EXTRA_BUFS = 8

EPS = 1e-5


def _build_program(repeats: int = 1) -> bass.Bass:
    nc = bacc.Bacc("TRN2", target_bir_lowering=False, debug=False)

    xd = nc.dram_tensor("x", [NPC, C, H, W], F32, kind="ExternalInput").ap()
    w1t_d = nc.dram_tensor("w1t", [CH, 128, MIP], F32, kind="ExternalInput").ap()
    bns_d = nc.dram_tensor("bns", [MIP, 1], F32, kind="ExternalInput").ap()
    bnb_d = nc.dram_tensor("bnb", [MIP, 1], F32, kind="ExternalInput").ap()
    wht_d = nc.dram_tensor("wht6", [MIP, C], F32, kind="ExternalInput").ap()
    chb_d = nc.dram_tensor("chb", [CH, 128, 1], F32, kind="ExternalInput").ap()
    fct_d = nc.dram_tensor("fct", [CH, 128, C], F32, kind="ExternalInput").ap()
    fcb_d = nc.dram_tensor("fcb", [CH, 128, 1], F32, kind="ExternalInput").ap()
    outd = nc.dram_tensor("out", [NPC, C, H, W], F32, kind="ExternalOutput").ap()

    with tile.TileContext(nc) as tc, ExitStack() as ctx:
        consts = ctx.enter_context(tc.tile_pool(name="consts", bufs=1))
        xt_pool = ctx.enter_context(tc.tile_pool(name="xt", bufs=HH * CH + XT_EXTRA_BUFS))
        small = ctx.enter_context(tc.tile_pool(name="small", bufs=3))
        psum = ctx.enter_context(tc.tile_pool(name="psum", bufs=2, space="PSUM"))

        # ---- load replicated parameters once ----
        w1t_sb = consts.tile([128, CH, MIP], F32, tag="w1t")
        fct_sb = consts.tile([128, CH, C], F32, tag="fct")
        chb_sb = consts.tile([128, CH, 1], F32, tag="chb")
        fcb_sb = consts.tile([128, CH, 1], F32, tag="fcb")
        for ch in range(CH):
            nc.sync.dma_start(out=w1t_sb[:, ch, :], in_=w1t_d[ch])
            nc.sync.dma_start(out=fct_sb[:, ch, :], in_=fct_d[ch])
            nc.sync.dma_start(out=chb_sb[:, ch, :], in_=chb_d[ch])
            nc.sync.dma_start(out=fcb_sb[:, ch, :], in_=fcb_d[ch])
        wht_sb = consts.tile([MIP, C], F32, tag="wht")
        nc.sync.dma_start(out=wht_sb[:], in_=wht_d[:])
        bns_sb = consts.tile([MIP, 1], F32, tag="bns")
        nc.sync.dma_start(out=bns_sb[:], in_=bns_d[:])
        bnb_sb = consts.tile([MIP, 1], F32, tag="bnb")
        nc.sync.dma_start(out=bnb_sb[:], in_=bnb_d[:])
        three_sb = consts.tile([MIP, 1], F32, tag="three")
        nc.vector.memset(three_sb[:], 3.0)

        for s in [s for _ in range(repeats) for s in range(NPC)]:
            # ---- load sample, 4 tiles of [128c, HT, 128w] ----
            xts = {}
            for ch in range(CH):
                for hh in range(HH):
                    xt = xt_pool.tile([128, HT, W], F32, tag="xt")
                    nc.sync.dma_start(
                        out=xt[:],
                        in_=xd[s, ch * 128:(ch + 1) * 128, hh * HT:(hh + 1) * HT, :],
                    )
                    xts[ch, hh] = xt

            # ---- width pools (DVE) ----
            xh_sum, xh_max = {}, {}
            for ch in range(CH):
                hs = small.tile([128, H], F32, tag="xh_sum")
                hm = small.tile([128, H], F32, tag="xh_max")
                for hh in range(HH):
                    sl = slice(hh * HT, (hh + 1) * HT)
                    nc.vector.reduce_sum(hs[:, sl], xts[ch, hh][:], axis=AX.X)
                    nc.vector.reduce_max(hm[:, sl], xts[ch, hh][:], axis=AX.X)
                xh_sum[ch], xh_max[ch] = hs, hm

            # x_h = wsum/W + wmax ; g = gsum/(H*W) + gmax
            xh, g = {}, {}
            for ch in range(CH):
                t = small.tile([128, H], F32, tag="xh")
                nc.vector.scalar_tensor_tensor(
                    t[:], in0=xh_sum[ch][:], scalar=1.0 / W, in1=xh_max[ch][:],
                    op0=ALU.mult, op1=ALU.add,
                )
                xh[ch] = t
                gs = small.tile([128, 1], F32, tag="gsum")
                gm = small.tile([128, 1], F32, tag="gmax")
                gt = small.tile([128, 1], F32, tag="g")
                nc.vector.reduce_sum(gs[:], xh_sum[ch][:], axis=AX.X)
                nc.vector.reduce_max(gm[:], xh_max[ch][:], axis=AX.X)
                nc.vector.scalar_tensor_tensor(
                    gt[:], in0=gs[:], scalar=1.0 / (H * W), in1=gm[:],
                    op0=ALU.mult, op1=ALU.add,
                )
                g[ch] = gt

            # ---- conv1 + BN + h_swish (tiny, [MIP, H]) ----
            y_ps = psum.tile([MIP, H], F32, tag="y_ps")
            for ch in range(CH):
                nc.tensor.matmul(
                    y_ps[:], lhsT=w1t_sb[:, ch, :], rhs=xh[ch][:],
                    start=(ch == 0), stop=(ch == CH - 1),
                )
            y_sb = small.tile([MIP, H], F32, tag="y")
            nc.scalar.activation(
                y_sb[:], y_ps[:], AF.Identity, bias=bnb_sb[:], scale=bns_sb[:],
            )
            t_sb = small.tile([MIP, H], F32, tag="t")
            nc.scalar.activation(t_sb[:], y_sb[:], AF.Relu, bias=three_sb[:])
            nc.vector.tensor_scalar_min(t_sb[:], t_sb[:], 6.0)
            y2 = small.tile([MIP, H], F32, tag="y2")
            nc.vector.tensor_mul(y2[:], y_sb[:], t_sb[:])

            # ---- a_h = sigmoid(conv_h/6 @ y2 + b) ; per-chunk [128, H] ----
            a2 = {}
            for ch in range(CH):
                aps = psum.tile([128, H], F32, tag="a_ps")
                nc.tensor.matmul(
                    aps[:], lhsT=wht_sb[:, ch * 128:(ch + 1) * 128], rhs=y2[:],
                )
                a = small.tile([128, H], F32, tag="a")
                nc.scalar.activation(a[:], aps[:], AF.Sigmoid, bias=chb_sb[:, ch, :])
                a2[ch] = a

            # ---- ca = sigmoid(fc @ g + b) ; per-chunk [128, 1] ----
            for ch in range(CH):
                cps = psum.tile([128, 1], F32, tag="ca_ps")
                for j in range(CH):
                    nc.tensor.matmul(
                        cps[:], lhsT=fct_sb[:, j, ch * 128:(ch + 1) * 128],
                        rhs=g[j][:], start=(j == 0), stop=(j == CH - 1),
                    )
                casb = small.tile([128, 1], F32, tag="ca")
                nc.scalar.activation(casb[:], cps[:], AF.Sigmoid, bias=fcb_sb[:, ch, :])
                # fold channel attention into a_h: a2 = a_h * ca  (per-partition)
                nc.vector.tensor_scalar_mul(a2[ch][:], a2[ch][:], casb[:])

            # ---- out = x * a2[c,h]  (per-h scale; ACT with optional DVE share)
            dve_frac = MUL_DVE_FRAC.get(s % NPC, 0.0)
            thresh = int(round(dve_frac * 10))
            for ch in range(CH):
                for hh in range(HH):
                    xt = xts[ch, hh]
                    for j in range(HT):
                        h = hh * HT + j
                        if (j * 7919 + ch * 13 + hh * 3) % 10 < thresh:
                            nc.vector.tensor_scalar_mul(
                                xt[:, j, :], xt[:, j, :], a2[ch][:, h:h + 1],
                            )
                        else:
                            nc.scalar.mul(
                                xt[:, j, :], xt[:, j, :], a2[ch][:, h:h + 1],
                            )
                    nc.sync.dma_start(
                        out=outd[s, ch * 128:(ch + 1) * 128, hh * HT:(hh + 1) * HT, :],
                        in_=xt[:],
                    )
    nc.compile()
    return nc


_NC_CACHE = {}


def _get_program(repeats: int = 1) -> bass.Bass:
    if repeats not in _NC_CACHE:
        _NC_CACHE[repeats] = _build_program(repeats)
    return _NC_CACHE[repeats]


def _prep_in_maps(inputs: dict) -> list:
    f = lambda a: np.ascontiguousarray(np.asarray(a, dtype=np.float32))
    x = f(inputs["x"])
    conv1_w = f(inputs["conv1_w"])
    conv1_b = f(inputs["conv1_b"])
    bn_gamma = f(inputs["bn_gamma"])
    bn_beta = f(inputs["bn_beta"])
    bn_mean = f(inputs["bn_mean"])
    bn_var = f(inputs["bn_var"])
    conv_h_w = f(inputs["conv_h_w"])
    conv_h_b = f(inputs["conv_h_b"])
    fc_w = f(inputs["fc_w"])
    fc_b = f(inputs["fc_b"])

    # Host-side folds (all tiny):
    #   BN(y) = y*bns + bnb  with bns = gamma/sqrt(var+eps), bnb = beta - mean*bns
    #   ... but conv1 bias must be applied before BN: fold conv1_b into bnb.
    bns = bn_gamma / np.sqrt(bn_var + EPS)
    bnb = bn_beta + (conv1_b - bn_mean) * bns
    w1t = np.ascontiguousarray(conv1_w.T).reshape(CH, 128, MIP)
    wht6 = np.ascontiguousarray(conv_h_w.T) / 6.0           # [MIP, C]
    chb = conv_h_b.reshape(CH, 128, 1)
    fct = np.ascontiguousarray(fc_w.T).reshape(CH, 128, C)  # fct[j, c] = fc_w[c, j]
    fcb = fc_b.reshape(CH, 128, 1)

    shared = {
        "w1t": w1t.astype(np.float32),
        "bns": bns.reshape(MIP, 1).astype(np.float32),
        "bnb": bnb.reshape(MIP, 1).astype(np.float32),
        "wht6": wht6.astype(np.float32),
        "chb": chb.astype(np.float32),
        "fct": fct.astype(np.float32),
        "fcb": fcb.astype(np.float32),
    }
    return [
        {"x": np.ascontiguousarray(x[i * NPC:(i + 1) * NPC]), **shared}
        for i in range(N_CORES)
    ]


def _run(inputs: dict, trace: bool = False, repeats: int = 1):
    nc = _get_program(repeats)
    in_maps = _prep_in_maps(inputs)
    res = run_bass_kernel_spmd(nc, in_maps, list(range(N_CORES)), trace=trace)
    out = np.concatenate([res.results[i]["out"] for i in range(N_CORES)], axis=0)
    return out, res


def kernel(**inputs) -> np.ndarray:
    out, _ = _run(inputs)
    return out


if __name__ == "__main__":
    # smoke test with random data through the simulator-free path
    rng = np.random.default_rng(0)
    ins = {
        "x": rng.standard_normal((N, C, H, W), dtype=np.float32),
        "conv1_w": rng.standard_normal((MIP, C), dtype=np.float32) * 0.05,
        "conv1_b": rng.standard_normal((MIP,), dtype=np.float32) * 0.05,
        "bn_gamma": np.ones((MIP,), np.float32),
        "bn_beta": np.zeros((MIP,), np.float32),
        "bn_mean": rng.standard_normal((MIP,), dtype=np.float32) * 0.1,
        "bn_var": rng.random((MIP,), dtype=np.float32) * 0.5 + 0.5,
        "conv_h_w": rng.standard_normal((C, MIP), dtype=np.float32) * 0.05,
        "conv_h_b": rng.standard_normal((C,), dtype=np.float32) * 0.05,
        "fc_w": rng.standard_normal((C, C), dtype=np.float32) * 0.05,
        "fc_b": rng.standard_normal((C,), dtype=np.float32) * 0.05,
    }
    out = kernel(**ins)
    print("out", out.shape, out.dtype, float(np.abs(out).max()))
